# revision 10
# baseline (speedup 1.0000x reference)
"""CTC loss (keras ctc_batch_cost semantics) on 8 Trainium2 NeuronCores.

Data parallel: 32 examples per core. The sequential alpha recurrence runs in
the probability domain with periodic rescaling (every 32 steps).

Upload is minimized (the real device is pseudo-DMA byte-bound): instead of
host-precomputed per-state coefficients ([97, *] streams), each core receives
only the 49 informative probabilities per (example, timestep) - the 48 label
probs plus the blank prob - quantized to fp8_e4m3:

    ysm[j, t, b] = 512 * (y_pred[b, t, idx_b[j]] + EPS),  idx_b = [labels, BLANK]

i.e. 0.80 MB per core vs 7.9 MB for the previous coefficient upload. On
device a fixed 0/1 matrix Eq (fp8) expands the 49 rows to the 97 CTC states
(odd state 2j+1 <- row j, even states <- blank row 48) via 32 PSUM-chunked
matmuls, and the per-example skip mask md2 (uploaded inside aux) forms the
second coefficient stream r = md2 * q, giving a bf16 SBUF tensor
qr[97, T, 2, n] that never touches HBM.

Recurrence (z-stream form, z_t = M(q_t*z_{t-1}), answer = sel.(q_{T-1}*z_{T-2})):
    uv = [q_t | r_t] * dup(z_{t-1})      one fused DVE multiply per step/group
    z_t = W1^T uv.u + W2^T uv.v          two PSUM-accumulating matmuls
with states on partitions ([97, batch]); W1 = I+S1, W2 = S2 are shared 0/1
weights. Every 32 steps uv is renormalized by 1/colsum(u) (colsum + broadcast
both done on the PE; factors log-accumulated on the host... on device).

loss = -(log(u_T[95]+u_T[96]) + sum_j log(c_j) - T*log(512)).

End-to-end numpy emulation of this exact scheme (fp8 coefficients, bf16
recurrence rounding) matches the jax reference to 1.6e-3 max rel err.

NOTE on DMA structure: this walrus build lowers DMA/memset to pseudo-DMA
instructions that accept at most ONE sync-wait command, so the program keeps
all loads write-once/dependency-free and budgets < 8 DMA-lowered instructions
before the single (dependency-carrying) loss store.
"""
import os
import sys
import numpy as np

for _p in ("/opt/trn_rl_repo", "/root/.axon_site/_ro/trn_rl_repo"):
    if os.path.isdir(_p) and _p not in sys.path:
        sys.path.insert(0, _p)

import ml_dtypes  # noqa: E402
import concourse.bass as bass  # noqa: E402
import concourse.bacc as bacc  # noqa: E402
import concourse.mybir as mybir  # noqa: E402
import concourse.tile as tile  # noqa: E402
from concourse.bass_utils import run_bass_kernel_spmd  # noqa: E402

BF = ml_dtypes.bfloat16
F8 = ml_dtypes.float8_e4m3
F32 = np.float32

B, T, L, C = 256, 512, 48, 512
S = 2 * L + 1          # 97
J = L + 1              # 49 uploaded rows: 48 labels + blank
BLANK = C - 1
EPS = 1e-7
ZQ = 512.0             # per-step scale folded into the coefficients
NCORES = 8
BPC = B // NCORES      # 32 examples per core
RESC = 32              # rescale interval (steps)
NG = 2                 # recurrence groups (pipeline DVE vs PE)
TC = 16                # expansion chunk timesteps (TC*BPC f32 = one PSUM bank)
NDMA = 4               # ysm upload pieces (t-sliced, stream compute behind)

# aux column layout
_W1_OFF = 0
_W2_OFF = S
_ONES_OFF = 2 * S
_SEL_OFF = 2 * S + 1
_E01_OFF = 2 * S + 2
_BC1_OFF = 2 * S + 3
_MD2_OFF = 3 * S + 3
_NCOL = 3 * S + 3 + BPC


def _resc_ts(Tt):
    return [t for t in range(RESC, Tt - 1, RESC)]


# ---------------------------------------------------------------------------
# host-side precompute
# ---------------------------------------------------------------------------

def host_ysm(y_true, y_pred):
    """[J, Tt, n] fp8: q[j,t,b] = ZQ*(y_pred[b,t,idx[b,j]] + EPS)."""
    lab = np.asarray(y_true).astype(np.int64)
    y = np.asarray(y_pred, dtype=F32)
    n = lab.shape[0]
    idx = np.concatenate([lab, np.full((n, 1), BLANK, np.int64)], axis=1)
    g = np.take_along_axis(y, idx[:, None, :], axis=2)      # [n, Tt, J]
    g = ZQ * (g + EPS)
    return np.ascontiguousarray(g.transpose(2, 1, 0)).astype(F8)


def host_aux(y_true):
    """aux [S, _NCOL] bf16: W1 | W2 | ones | sel | e01 | bc1 | md2."""
    lab = np.asarray(y_true).astype(np.int64)
    n = lab.shape[0]
    aux = np.zeros((S, _NCOL), dtype=F32)
    ss = np.arange(S)
    # W1^T: out[s] = u[s] + u[s-1]  ->  W1[i, s] = 1 iff i in {s, s-1}
    aux[ss, _W1_OFF + ss] = 1.0
    aux[ss[1:] - 1, _W1_OFF + ss[1:]] = 1.0
    # W2^T: out[s] = v[s-2]        ->  W2[i, s] = 1 iff i == s-2
    aux[ss[2:] - 2, _W2_OFF + ss[2:]] = 1.0
    aux[:, _ONES_OFF] = 1.0
    aux[S - 2:S, _SEL_OFF] = 1.0
    aux[0:2, _E01_OFF] = 1.0
    aux[0, _BC1_OFF:_BC1_OFF + S] = 1.0
    # md2[s, b] = mask[s+2, b]; mask[s] = skip-transition-allowed into s
    ext = np.full((n, S), BLANK, dtype=np.int64)
    ext[:, 1::2] = lab
    m = np.zeros((n, S), dtype=F32)
    m[:, 1] = 1.0
    odd = np.arange(3, S, 2)
    m[:, odd] = (ext[:, odd] != ext[:, odd - 2]).astype(F32)
    aux[:S - 2, _MD2_OFF:_MD2_OFF + n] = m[:, 2:].T
    return aux.astype(BF)


def host_eqw():
    """Eq [J, S] fp8: odd s=2j+1 <- row j, even s <- blank row J-1."""
    eq = np.zeros((J, S), dtype=F32)
    for s in range(S):
        eq[(s - 1) // 2 if s % 2 == 1 else J - 1, s] = 1.0
    return eq.astype(F8)


# ---------------------------------------------------------------------------
# device program
# ---------------------------------------------------------------------------

def build_bass(n_ex=BPC, Tt=T):
    dtb = mybir.dt.bfloat16
    dt8 = mybir.dt.float8e4
    dtf = mybir.dt.float32
    resc = _resc_ts(Tt)
    ncs = len(resc) + 1
    gsz = n_ex // NG
    gsl = [slice(g * gsz, (g + 1) * gsz) for g in range(NG)]

    nc = bacc.Bacc()
    ysm_d = nc.dram_tensor("ysm", [J, Tt, n_ex], dt8, kind="ExternalInput")
    aux_d = nc.dram_tensor("aux", [S, _NCOL], dtb, kind="ExternalInput")
    eqw_d = nc.dram_tensor("eqw", [J, S], dt8, kind="ExternalInput")
    loss_d = nc.dram_tensor("loss", [n_ex, 1], dtf, kind="ExternalOutput")

    with tile.TileContext(nc) as tc:
        with tc.tile_pool(name="persist", bufs=1) as persist:
            ysm_t = persist.tile([J, Tt, n_ex], dt8, tag="ysm")
            aux_t = persist.tile([S, _NCOL], dtb, tag="aux")
            eqw_t = persist.tile([J, S], dt8, tag="eqw")
            qr = persist.tile([S, Tt, 2, n_ex], dtb, tag="qr")
            cbuf = persist.tile([1, ncs, n_ex], dtf, tag="cbuf")
            logbuf = persist.tile([1, ncs, n_ex], dtf, tag="logbuf")
            rscale = persist.tile([1, n_ex], dtb, tag="rscale")
            llsum = persist.tile([1, n_ex], dtf, tag="llsum")
            lossb = persist.tile([1, n_ex], dtf, tag="lossb")
            e01f = persist.tile([S, 1], dtf, tag="e01f")

            # aux/eqw first, then ysm in t-order pieces so the expansion and
            # the (t-sequential) recurrence stream behind the upload.
            nc.gpsimd.dma_start(aux_t[:], aux_d[:])
            nc.gpsimd.dma_start(eqw_t[:], eqw_d[:])
            tpp = Tt // NDMA
            for i in range(NDMA):
                nc.gpsimd.dma_start(ysm_t[:, i * tpp:(i + 1) * tpp, :],
                                    ysm_d[:, i * tpp:(i + 1) * tpp, :])

            w1 = aux_t[:, _W1_OFF:_W1_OFF + S]
            w2 = aux_t[:, _W2_OFF:_W2_OFF + S]
            ones_col = aux_t[:, _ONES_OFF:_ONES_OFF + 1]
            sel_col = aux_t[:, _SEL_OFF:_SEL_OFF + 1]
            e01_col = aux_t[:, _E01_OFF:_E01_OFF + 1]
            bc1_row = aux_t[0:1, _BC1_OFF:_BC1_OFF + S]
            md2 = aux_t[:, _MD2_OFF:_MD2_OFF + n_ex]

            nc.scalar.copy(e01f[:], e01_col)

            # ---- expansion: qr[s, t, 0, b] = q, qr[s, t, 1, b] = md2*q ----
            with tc.tile_pool(name="pse", bufs=2, space="PSUM") as psE:
                for tlo in range(0, Tt, TC):
                    ps = psE.tile([S, TC, n_ex], dtf, tag="ps", name=f"ps{tlo}")
                    nc.tensor.matmul(ps[:], eqw_t[:], ysm_t[:, tlo:tlo + TC, :],
                                     start=True, stop=True)
                    nc.scalar.copy(qr[:, tlo:tlo + TC, 0, :], ps[:])
                    nc.vector.tensor_tensor(
                        qr[:, tlo:tlo + TC, 1, :], ps[:],
                        md2.unsqueeze(1).broadcast_to([S, TC, n_ex]),
                        mybir.AluOpType.mult)

            # ---- recurrence ----
            with (
                tc.tile_pool(name="uv", bufs=2) as uvP,
                tc.tile_pool(name="zp", bufs=2, space="PSUM") as zP,
                tc.tile_pool(name="csp", bufs=2, space="PSUM") as csP,
            ):
                uvt = [[uvP.tile([S, 2, gsz], dtb, tag=f"uv{g}{p}",
                                 name=f"uv{g}{p}") for p in range(2)]
                       for g in range(NG)]
                z_prev = [None] * NG
                for t in range(Tt - 1):
                    for g in range(NG):
                        uv = uvt[g][t % 2]
                        if t == 0:
                            nc.scalar.mul(uv[:], qr[:, t, :, gsl[g]],
                                          e01f[:])
                        else:
                            src = z_prev[g][:].unsqueeze(1)\
                                .broadcast_to([S, 2, gsz])
                            nc.vector.tensor_tensor(uv[:], src,
                                                    qr[:, t, :, gsl[g]],
                                                    mybir.AluOpType.mult)
                        if t in resc:
                            j = resc.index(t)
                            cs = csP.tile([1, gsz], dtf, tag=f"cs{g}",
                                          name=f"cs_{t}_{g}")
                            nc.tensor.matmul(cs[:], ones_col, uv[:, 0, :],
                                             start=True, stop=True)
                            with nc.allow_low_precision(
                                    reason="bf16 rescale factor; rounding "
                                           "cancels via logged f32 colsum"):
                                nc.vector.reciprocal(rscale[:, gsl[g]],
                                                     cs[:])
                            nc.scalar.copy(cbuf[:, j, gsl[g]], cs[:])
                            rb = csP.tile([S, gsz], dtf, tag=f"cs{g}",
                                          name=f"rb_{t}_{g}")
                            nc.tensor.matmul(rb[:], bc1_row,
                                             rscale[:, gsl[g]],
                                             start=True, stop=True)
                            nc.vector.tensor_tensor(
                                uv[:], uv[:],
                                rb[:].unsqueeze(1).broadcast_to([S, 2, gsz]),
                                mybir.AluOpType.mult)
                        z = zP.tile([S, gsz], dtf, tag=f"z{g}",
                                    name=f"z_{t}_{g}")
                        nc.tensor.matmul(z[:], w1, uv[:, 0, :],
                                         start=True, stop=False)
                        nc.tensor.matmul(z[:], w2, uv[:, 1, :],
                                         start=False, stop=True)
                        z_prev[g] = z

                # final step t = Tt-1: u only; answer = sel . u
                for g in range(NG):
                    uvf = uvP.tile([S, 1, gsz], dtb, tag=f"uv{g}0",
                                   name=f"uvf{g}")
                    nc.vector.tensor_tensor(
                        uvf[:], z_prev[g][:].unsqueeze(1),
                        qr[:, Tt - 1, 0:1, gsl[g]], mybir.AluOpType.mult)
                    fin = csP.tile([1, gsz], dtf, tag=f"cs{g}",
                                   name=f"fin{g}")
                    nc.tensor.matmul(fin[:], sel_col, uvf[:, 0, :],
                                     start=True, stop=True)
                    nc.scalar.copy(cbuf[:, ncs - 1, gsl[g]], fin[:])

            nc.scalar.activation(logbuf[:], cbuf[:],
                                 mybir.ActivationFunctionType.Ln)
            nc.vector.tensor_reduce(
                llsum[:], logbuf[:].rearrange("p j b -> p b j"),
                mybir.AxisListType.X, mybir.AluOpType.add)
            for _ in range(2):
                nc.scalar.activation(lossb[:], llsum[:],
                                     mybir.ActivationFunctionType.Copy,
                                     bias=float(Tt * np.log(ZQ)), scale=-1.0)
            nc.gpsimd.dma_start(loss_d[:, 0].unsqueeze(0), lossb[0:1, :])
    nc.compile()
    return nc


# ---------------------------------------------------------------------------
# entry point
# ---------------------------------------------------------------------------

_CACHE = {}


def _get_nc():
    if "nc" not in _CACHE:
        _CACHE["nc"] = build_bass()
    return _CACHE["nc"]


def make_in_maps(y_true, y_pred):
    y_true = np.asarray(y_true)
    y_pred = np.asarray(y_pred, dtype=F32)
    eqw = host_eqw()
    in_maps = []
    for core in range(NCORES):
        sl = slice(core * BPC, (core + 1) * BPC)
        in_maps.append({
            "ysm": host_ysm(y_true[sl], y_pred[sl]),
            "aux": host_aux(y_true[sl]),
            "eqw": eqw,
        })
    return in_maps


def kernel(y_true, y_pred):
    nc = _get_nc()
    in_maps = make_in_maps(y_true, y_pred)
    res = run_bass_kernel_spmd(nc, in_maps, list(range(NCORES)))
    out = np.concatenate([res.results[c]["loss"] for c in range(NCORES)],
                         axis=0)
    return out.astype(F32)
